# revision 18
# baseline (speedup 1.0000x reference)
"""Causal self-attention (B=8, T=1024, C=768, H=8 heads) for 8 TRN2 NeuronCores.

Strategy: pure data parallelism — one batch element per core. Each core runs an
identical Bass/Tile program computing the full attention block for its batch
element; weights are replicated. No collectives.

Per-core pipeline:
  1. x [T,C] -> x^T [C,T] via PE transposes in f32r (contraction dim must be on
     partitions), cast to the bf16 xT tiles on copyback.
  2. v = x @ W_v + b_v in [token, feat] layout (f32r matmuls), stored per
     128-token block as v_aug [128, 8*97] bf16: per head 96 value columns plus
     a ones column (the ones column makes the P@V matmul also produce the
     softmax denominator).
  3. Per head h: S(h) -> qk-proj(h+1) -> PV(h) -> tail(h).  The qk projection
     (f32r) sits between the S matmuls and the P@V matmuls so the scalar
     engine's exp chain for head h overlaps PE work instead of stalling it.
     S and PV run in bf16 (qT/kT/p_t/v_aug bf16): no fp32r narrow-output
     penalty and half the SBUF traffic.
  4. Softmax denominator: PV row 96 = sum(exp); reciprocal via the DVE
     (reciprocal_approx_fast) instead of Ln/Exp on the scalar engine; the
     per-query broadcast goes through a DRAM staging row (SBUF->SBUF DMA
     cannot do 0-stride broadcast, DRAM->SBUF can).
  5. out = y @ W_proj + b_proj with the feature-packed y^T (f32r) as lhsT.
"""
import sys

sys.path.insert(0, "/opt/trn_rl_repo")

import numpy as np

T, C, H, D = 1024, 768, 8, 96
C3 = 3 * C
P = 128
NT = T // P   # 8 token blocks
NCB = C // P  # 6 feature blocks
DA = D + 1    # 97: head dim + denominator column

_CACHE = {}


def _build():
    import concourse.bacc as bacc
    import concourse.mybir as mybir
    import concourse.tile as tile
    from concourse.masks import make_identity

    F32 = mybir.dt.float32
    F32R = mybir.dt.float32r
    BF16 = mybir.dt.bfloat16
    Exp = mybir.ActivationFunctionType.Exp
    is_ge = mybir.AluOpType.is_ge
    SCALE = 1.0 / float(np.sqrt(D))

    nc = bacc.Bacc("TRN2", target_bir_lowering=False, debug=False, num_devices=8)
    x_d = nc.dram_tensor("x", [T, C], F32, kind="ExternalInput").ap()
    wa_d = nc.dram_tensor("W_attn", [C, C3], F32, kind="ExternalInput").ap()
    ba_d = nc.dram_tensor("b_attn", [C3], F32, kind="ExternalInput").ap()
    wp_d = nc.dram_tensor("W_proj", [C, C], F32, kind="ExternalInput").ap()
    bp_d = nc.dram_tensor("b_proj", [C], F32, kind="ExternalInput").ap()
    out_d = nc.dram_tensor("out", [T, C], F32, kind="ExternalOutput").ap()

    with tile.TileContext(nc) as tc:
        with tc.tile_pool(name="const", bufs=1) as const_p, \
             tc.tile_pool(name="vp", bufs=1) as v_p, \
             tc.tile_pool(name="qkt", bufs=4) as qk_p, \
             tc.tile_pool(name="yt", bufs=1) as yT_p, \
             tc.tile_pool(name="sm", bufs=2) as sm_p, \
             tc.tile_pool(name="ob", bufs=2) as o_p, \
             tc.tile_pool(name="pp", bufs=8) as p_p, \
             tc.tile_pool(name="ps", bufs=1, space="PSUM") as ps:
            # f32 constants: cols 0:128 identity (PE transposes), 128:144
            # per-head q/k bias columns (rows 0:96)
            constF = const_p.tile([P, 144], F32, name="constF")
            ident = constF[:, 0:P]
            b_qk = constF[0:D, P:P + 16]
            # bf16 constants: tri mask + ones columns for v_aug
            constB = const_p.tile([P, P + H], BF16, name="constB")
            tri = constB[:, 0:P]
            ones8 = constB[:, P:P + H]
            bv_bc = const_p.tile([P, C], F32, name="bv_bc")
            vA = [v_p.tile([P, DA * H], BF16, name=f"vA{t}") for t in range(NT)]
            yTp = [yT_p.tile([P, T], F32R, name=f"yTp{cb}") for cb in range(NCB)]
            # DRAM staging for the reciprocal rows (SBUF->SBUF DMA cannot do
            # 0-stride broadcast, DRAM->SBUF can)
            rc_dram = nc.dram_tensor("rc_stage", [H, T], F32,
                                     kind="Internal").ap()

            with tc.tile_pool(name="xT", bufs=1) as xT_p, \
                 tc.tile_pool(name="wqk", bufs=1) as wqk_p:
                xT = [xT_p.tile([P, T], F32R, name=f"xT{cb}") for cb in range(NCB)]

                # ---- x^T transposes + v projection (scoped W_v / x loads) ----
                with tc.tile_pool(name="xl", bufs=4) as x_p, \
                     tc.tile_pool(name="wv", bufs=1) as wv_p:
                    # DMA issue order = sync-queue order: first x (gates the
                    # PE transposes), then W_v, then the bias broadcasts /
                    # scatter, then W_qk / W_proj below.
                    x_ts = []
                    for tb in range(4):
                        x_t = x_p.tile([P, C], F32, name="x_t")
                        nc.sync.dma_start(x_t[:], x_d[tb * P:(tb + 1) * P, :])
                        x_ts.append(x_t)
                    wv = []
                    for cb in range(NCB):
                        w = wv_p.tile([P, C], F32R, name=f"wv{cb}")
                        nc.sync.dma_start(w[:], wa_d[cb * P:(cb + 1) * P,
                                                     2 * C:3 * C].bitcast(F32R))
                        wv.append(w)
                    nc.sync.dma_start(
                        bv_bc[:],
                        ba_d.unsqueeze(0)[:, 2 * C:3 * C].partition_broadcast(P).squeeze(1))
                    for tb in range(4, NT):
                        x_t = x_p.tile([P, C], F32, name="x_t")
                        nc.sync.dma_start(x_t[:], x_d[tb * P:(tb + 1) * P, :])
                        x_ts.append(x_t)
                    nc.sync.dma_start(b_qk,
                                      ba_d.rearrange("(a b) -> b a", b=D)[:, 0:16])

                    # constants built while DMAs stream.  The tri mask is
                    # built in f32 (gpsimd affine_select) and rounded to the
                    # bf16 tile with a DVE copy.
                    make_identity(nc, ident)
                    tri_f = x_p.tile([P, P], F32, name="tri_f",
                                     tag="trif", bufs=1)
                    nc.gpsimd.memset(tri_f, 1.0)
                    nc.gpsimd.affine_select(
                        out=tri_f, in_=tri_f, compare_op=is_ge, fill=0.0,
                        base=0, pattern=[[1, P]], channel_multiplier=-1)
                    nc.vector.tensor_copy(tri, tri_f[:])
                    nc.vector.memset(ones8, 1.0)

                    # dummy transposes keep the PE busy while x streams in so
                    # the HAM clock-gate is already released (2.4 GHz) when
                    # the real work starts
                    for w_i in range(10):
                        warm_ps = ps.tile([P, 512], F32, name="warm_ps",
                                          tag="big", bufs=3)
                        for k in range(4):
                            nc.tensor.transpose(warm_ps[:, k * P:(k + 1) * P],
                                                ident, ident)

                    for jt in range(2):
                        for cb in range(NCB):
                            tr_ps = ps.tile([P, 512], F32, name="tr_ps",
                                            tag="big", bufs=3)
                            for k in range(4):
                                nc.tensor.transpose(
                                    tr_ps[:, k * P:(k + 1) * P],
                                    x_ts[4 * jt + k][:, cb * P:(cb + 1) * P],
                                    ident)
                            nc.vector.tensor_copy(xT[cb][:, jt * 512:(jt + 1) * 512],
                                                  tr_ps[:])
                        # v projection for this half's token blocks
                        for tb in range(4 * jt, 4 * jt + 4):
                            v_ps = ps.tile([P, C], F32, name="v_ps", tag="big", bufs=3)
                            for cb in range(NCB):
                                lhsT = xT[cb][:, tb * P:(tb + 1) * P]
                                nc.tensor.matmul(v_ps[:, 0:512], lhsT,
                                                 wv[cb][:, 0:512],
                                                 start=(cb == 0), stop=(cb == NCB - 1))
                                nc.tensor.matmul(v_ps[:, 512:C], lhsT,
                                                 wv[cb][:, 512:C],
                                                 start=(cb == 0), stop=(cb == NCB - 1))
                            for h in range(H):
                                nc.vector.tensor_add(vA[tb][:, DA * h:DA * h + D],
                                                     v_ps[:, D * h:D * h + D],
                                                     bv_bc[:, D * h:D * h + D])
                            # ones columns at local col 96 of each head's group
                            nc.vector.tensor_copy(vA[tb][:, D::DA], ones8)

                # ---- per-head attention ----
                wqk = []
                for cb in range(NCB):
                    w = wqk_p.tile([P, 2 * C], F32R, name=f"wqk{cb}")
                    nc.sync.dma_start(w[:], wa_d[cb * P:(cb + 1) * P,
                                                 0:2 * C].bitcast(F32R))
                    wqk.append(w)
                # W_proj loads go into the space freed by the wv/xl pools so
                # they complete long before the projection needs them
                wp_p = tc.alloc_tile_pool(name="wp", bufs=1)
                bp_bc = wp_p.tile([P, C], F32, name="bp_bc", tag="bpbc", bufs=1)
                nc.sync.dma_start(
                    bp_bc[:], bp_d.unsqueeze(0).partition_broadcast(P).squeeze(1))
                wp = []
                for cb in range(NCB):
                    w = wp_p.tile([P, C], F32R, name=f"wp{cb}")
                    nc.sync.dma_start(w[:], wp_d[cb * P:(cb + 1) * P, :].bitcast(F32R))
                    wp.append(w)

                Identity = mybir.ActivationFunctionType.Identity

                def emit_qkproj(h, qT, kT):
                    # qT/kT = (x @ W_{q,k} + b)^T in [d, token] bf16 layout.
                    # The q copyback runs on the DVE, the k copyback on the
                    # scalar engine (Identity + per-partition bias): balances
                    # the two engines' per-head load.
                    for dst, off, bcol, eng in (
                            (qT, D * h, b_qk[:, h:h + 1], "v"),
                            (kT, C + D * h, b_qk[:, 8 + h:9 + h], "s")):
                        qk_ps = ps.tile([D, T], F32, name="qk_ps", tag="big", bufs=3)
                        for jt in range(2):
                            sl = slice(jt * 512, (jt + 1) * 512)
                            for cb in range(NCB):
                                nc.tensor.matmul(qk_ps[:, sl],
                                                 wqk[cb][:, off:off + D],
                                                 xT[cb][:, sl],
                                                 start=(cb == 0), stop=(cb == NCB - 1))
                        if eng == "v":
                            nc.vector.tensor_scalar_add(dst[:], qk_ps[:], bcol)
                        else:
                            nc.scalar.activation(dst[:], qk_ps[:], Identity,
                                                 bias=bcol)

                def emit_tail(h, y_sb, bc_sb):
                    # part B for head h: reciprocal of the broadcast
                    # denominators (partition-parallel), normalize, scatter
                    # head rows into the feature-packed yT tiles (partition
                    # shift -> must go through DMA)
                    rcp = sm_p.tile([D, T], F32, name="rcp", tag="rcp", bufs=2)
                    nc.vector.reciprocal_approx_fast(rcp[:], bc_sb[:])
                    y_n = sm_p.tile([D, T], F32R, name="y_n", tag="yn", bufs=2)
                    nc.vector.tensor_mul(y_n[:], y_sb[0:D, :], rcp[:])
                    f0 = D * h
                    while f0 < D * (h + 1):
                        cb2, b0 = f0 // P, f0 % P
                        seg = min(P - b0, D * (h + 1) - f0)
                        nc.sync.dma_start(
                            yTp[cb2][b0:b0 + seg, :],
                            y_n[f0 - D * h:f0 - D * h + seg, :])
                        f0 += seg

                qT = qk_p.tile([D, T], BF16, name="qT", tag="qkt")
                kT = qk_p.tile([D, T], BF16, name="kT", tag="qkt")
                emit_qkproj(0, qT, kT)

                pending = None
                for h in range(H):
                    # ---- S^T blocks + exp ----
                    ptiles = []
                    for ib in range(NT):
                        q0 = P * ib
                        s_ps = ps.tile([P, T], F32, name="s_ps", tag="big", bufs=3)
                        kblk = kT[:, ib * P:(ib + 1) * P]
                        if q0 < 512:
                            nc.tensor.matmul(s_ps[:, q0:512], kblk,
                                             qT[:, q0:512], start=True, stop=True)
                        r0 = max(q0, 512)
                        nc.tensor.matmul(s_ps[:, r0:T], kblk,
                                         qT[:, r0:T], start=True, stop=True)
                        p_t = p_p.tile([P, T], BF16, name="p_t")
                        nc.scalar.activation(p_t[:, q0:T], s_ps[:, q0:T],
                                             Exp, scale=SCALE)
                        # zero the upper triangle of the diagonal 128-col block
                        nc.vector.tensor_mul(p_t[:, q0:q0 + P],
                                             p_t[:, q0:q0 + P], tri)
                        ptiles.append(p_t)

                    # ---- next head's q/k projection: PE work that overlaps
                    # the exp chain above ----
                    if h + 1 < H:
                        qT_n = qk_p.tile([D, T], BF16, name="qT", tag="qkt")
                        kT_n = qk_p.tile([D, T], BF16, name="kT", tag="qkt")
                        emit_qkproj(h + 1, qT_n, kT_n)

                    # ---- P@V with causal N-restriction; two bank-halves of
                    # q, each its own accumulation group.  Each half's tail
                    # part A (stage to SBUF + DRAM roundtrip for the
                    # partition broadcast) is emitted as soon as that half's
                    # accumulation group closes, so the last head's
                    # normalization chain starts mid-PV instead of after it.
                    # The reciprocal + normalization (part B) are deferred one
                    # head so no DVE op waits on the DMA roundtrip at the
                    # FIFO head. ----
                    y_l = ps.tile([DA, 512], F32, name="y_l", tag="yps", bufs=2)
                    y_r = ps.tile([DA, 512], F32, name="y_r", tag="yps", bufs=2)
                    y_sb = sm_p.tile([DA, T], F32, name="y_sb", tag="ysb", bufs=2)
                    bc_sb = sm_p.tile([D, T], F32, name="bc_sb", tag="bc", bufs=2)
                    row = rc_dram[h:h + 1, :]
                    for ib in range(4):
                        q0 = P * ib
                        va = vA[ib][:, DA * h:DA * h + DA]
                        nc.tensor.matmul(y_l[:, q0:512], va,
                                         ptiles[ib][:, q0:512],
                                         start=(ib == 0), stop=(ib == 3))
                        nc.tensor.matmul(y_r[:], va, ptiles[ib][:, 512:T],
                                         start=(ib == 0), stop=False)
                    nc.vector.tensor_copy(y_sb[:, 0:512], y_l[:])
                    nc.sync.dma_start(row[:, 0:512], y_sb[D:DA, 0:512])
                    nc.sync.dma_start(bc_sb[:, 0:512],
                                      row[:, 0:512].partition_broadcast(D).squeeze(1))
                    for ib in range(4, NT):
                        q0 = P * ib
                        va = vA[ib][:, DA * h:DA * h + DA]
                        nc.tensor.matmul(y_r[:, q0 - 512:512], va,
                                         ptiles[ib][:, q0:T],
                                         start=False, stop=(ib == NT - 1))
                    nc.vector.tensor_copy(y_sb[:, 512:T], y_r[:])
                    nc.sync.dma_start(row[:, 512:T], y_sb[D:DA, 512:T])
                    nc.sync.dma_start(bc_sb[:, 512:T],
                                      row[:, 512:T].partition_broadcast(D).squeeze(1))
                    if pending is not None:
                        emit_tail(*pending)
                    pending = (h, y_sb, bc_sb)
                    if h + 1 < H:
                        qT, kT = qT_n, kT_n

                # ---------------- projection ----------------
                # The first three token blocks emit their first 4
                # accumulation matmuls (features 0:512 = heads 0-5, long
                # since scattered) before the last head's part-B tail, so its
                # DMA roundtrip is covered by PE work; the cb4/cb5 matmuls
                # (heads 6-7 features) follow after.
                NEARLY = 3
                o_pss = []
                for tb in range(NEARLY):
                    o_ps = ps.tile([P, C], F32, name="o_ps", tag="big", bufs=3)
                    for cb in range(4):
                        nc.tensor.matmul(o_ps[:, 0:512],
                                         yTp[cb][:, tb * P:(tb + 1) * P],
                                         wp[cb][:, 0:512],
                                         start=(cb == 0), stop=False)
                    o_pss.append(o_ps)
                if pending is not None:
                    emit_tail(*pending)
                    pending = None
                for tb in range(NT):
                    if tb < NEARLY:
                        o_ps = o_pss[tb]
                        for cb in range(4, NCB):
                            nc.tensor.matmul(o_ps[:, 0:512],
                                             yTp[cb][:, tb * P:(tb + 1) * P],
                                             wp[cb][:, 0:512],
                                             start=False, stop=(cb == NCB - 1))
                    else:
                        o_ps = ps.tile([P, C], F32, name="o_ps", tag="big", bufs=3)
                        for cb in range(NCB):
                            nc.tensor.matmul(o_ps[:, 0:512],
                                             yTp[cb][:, tb * P:(tb + 1) * P],
                                             wp[cb][:, 0:512],
                                             start=(cb == 0), stop=(cb == NCB - 1))
                    o_sb = o_p.tile([P, C], F32, name="o_sb")
                    for cb in range(NCB):
                        nc.tensor.matmul(o_ps[:, 512:C],
                                         yTp[cb][:, tb * P:(tb + 1) * P],
                                         wp[cb][:, 512:C],
                                         start=(cb == 0), stop=(cb == NCB - 1))
                    nc.vector.tensor_add(o_sb[:, 0:512], o_ps[:, 0:512],
                                         bp_bc[:, 0:512])
                    nc.sync.dma_start(out_d[tb * P:(tb + 1) * P, 0:512],
                                      o_sb[:, 0:512])
                    nc.vector.tensor_add(o_sb[:, 512:C], o_ps[:, 512:C],
                                         bp_bc[:, 512:C])
                    nc.sync.dma_start(out_d[tb * P:(tb + 1) * P, 512:C],
                                      o_sb[:, 512:C])
                wp_p.release()

    # The act-table-load pass assigns each activation the first table set
    # containing its function; Exp and Identity must land in the same set or
    # every qk-copyback/exp switch costs a 1.3us table reload.  All our
    # functions live together in natural_log_exp_and_others, so hide the
    # other sets (keeping dict order — act_func_set_id is positional) during
    # this build.
    import concourse.hw_specs as hw_specs
    orig_tables = hw_specs.get_activation_tables

    def _tables(arch, *a, **kw):
        tabs = orig_tables(arch, *a, **kw)
        pref = "natural_log_exp_and_others"
        if pref not in tabs:
            return tabs
        return {k: (v if k == pref else type(v)()) for k, v in tabs.items()}

    import concourse.bacc as bacc_mod
    hw_specs.get_activation_tables = _tables
    bacc_orig = getattr(bacc_mod, "get_activation_tables", None)
    try:
        if bacc_orig is not None:
            bacc_mod.get_activation_tables = _tables
        nc.compile()
    finally:
        hw_specs.get_activation_tables = orig_tables
        if bacc_orig is not None:
            bacc_mod.get_activation_tables = bacc_orig
    return nc


def run(inputs, trace=False):
    import concourse.bass_utils as bass_utils

    nc = _CACHE.get("nc")
    if nc is None:
        nc = _CACHE["nc"] = _build()

    x = np.ascontiguousarray(inputs["x"], dtype=np.float32)
    wa = np.ascontiguousarray(inputs["W_attn"], dtype=np.float32)
    ba = np.ascontiguousarray(inputs["b_attn"], dtype=np.float32)
    wp = np.ascontiguousarray(inputs["W_proj"], dtype=np.float32)
    bp = np.ascontiguousarray(inputs["b_proj"], dtype=np.float32)
    B = x.shape[0]
    in_maps = [
        {"x": np.ascontiguousarray(x[b]), "W_attn": wa, "b_attn": ba,
         "W_proj": wp, "b_proj": bp}
        for b in range(B)
    ]
    res = bass_utils.run_bass_kernel_spmd(
        nc, in_maps, core_ids=list(range(B)), trace=trace)
    out = np.stack([r["out"] for r in res.results], axis=0)
    return out, res


def kernel(**inputs):
    out, _ = run(inputs, trace=False)
    return out


# revision 22
# speedup vs baseline: 1.0061x; 1.0061x over previous
"""Causal self-attention (B=8, T=1024, C=768, H=8 heads) for 8 TRN2 NeuronCores.

Strategy: pure data parallelism — one batch element per core. Each core runs an
identical Bass/Tile program computing the full attention block for its batch
element; weights are replicated. No collectives.

Per-core pipeline:
  1. x [T,C] -> x^T [C,T] via PE transposes in f32r (contraction dim must be on
     partitions), cast to the bf16 xT tiles on copyback.
  2. v = x @ W_v + b_v in [token, feat] layout (f32r matmuls), stored per
     128-token block as v_aug [128, 8*97] bf16: per head 96 value columns plus
     a ones column (the ones column makes the P@V matmul also produce the
     softmax denominator).
  3. Per head h: S(h) -> qk-proj(h+1) -> PV(h) -> tail(h).  The qk projection
     (f32r) sits between the S matmuls and the P@V matmuls so the scalar
     engine's exp chain for head h overlaps PE work instead of stalling it.
     S and PV run in bf16 (qT/kT/p_t/v_aug bf16): no fp32r narrow-output
     penalty and half the SBUF traffic.
  4. Softmax denominator: PV row 96 = sum(exp); reciprocal via the DVE
     (reciprocal_approx_fast) instead of Ln/Exp on the scalar engine; the
     per-query broadcast goes through a DRAM staging row (SBUF->SBUF DMA
     cannot do 0-stride broadcast, DRAM->SBUF can).
  5. out = y @ W_proj + b_proj with the feature-packed y^T (f32r) as lhsT.
"""
import sys

sys.path.insert(0, "/opt/trn_rl_repo")

import numpy as np

T, C, H, D = 1024, 768, 8, 96
C3 = 3 * C
P = 128
NT = T // P   # 8 token blocks
NCB = C // P  # 6 feature blocks
DA = D + 1    # 97: head dim + denominator column

_CACHE = {}


def _build():
    import concourse.bacc as bacc
    import concourse.mybir as mybir
    import concourse.tile as tile
    from concourse.masks import make_identity

    F32 = mybir.dt.float32
    F32R = mybir.dt.float32r
    BF16 = mybir.dt.bfloat16
    Exp = mybir.ActivationFunctionType.Exp
    is_ge = mybir.AluOpType.is_ge
    SCALE = 1.0 / float(np.sqrt(D))

    nc = bacc.Bacc("TRN2", target_bir_lowering=False, debug=False, num_devices=8)
    x_d = nc.dram_tensor("x", [T, C], F32, kind="ExternalInput").ap()
    wa_d = nc.dram_tensor("W_attn", [C, C3], F32, kind="ExternalInput").ap()
    ba_d = nc.dram_tensor("b_attn", [C3], F32, kind="ExternalInput").ap()
    wp_d = nc.dram_tensor("W_proj", [C, C], F32, kind="ExternalInput").ap()
    bp_d = nc.dram_tensor("b_proj", [C], F32, kind="ExternalInput").ap()
    out_d = nc.dram_tensor("out", [T, C], F32, kind="ExternalOutput").ap()

    with tile.TileContext(nc) as tc:
        with tc.tile_pool(name="const", bufs=1) as const_p, \
             tc.tile_pool(name="vp", bufs=1) as v_p, \
             tc.tile_pool(name="qkt", bufs=4) as qk_p, \
             tc.tile_pool(name="yt", bufs=1) as yT_p, \
             tc.tile_pool(name="sm", bufs=2) as sm_p, \
             tc.tile_pool(name="ob", bufs=2) as o_p, \
             tc.tile_pool(name="pp", bufs=8) as p_p, \
             tc.tile_pool(name="ps", bufs=1, space="PSUM") as ps:
            # f32 constants: cols 0:128 identity (PE transposes), 128:144
            # per-head q/k bias columns (rows 0:96)
            constF = const_p.tile([P, 144], F32, name="constF")
            ident = constF[:, 0:P]
            b_qk = constF[0:D, P:P + 16]
            # bf16 constants: tri mask + ones columns for v_aug
            constB = const_p.tile([P, P + H], BF16, name="constB")
            tri = constB[:, 0:P]
            ones8 = constB[:, P:P + H]
            bv_bc = const_p.tile([P, C], F32, name="bv_bc")
            vA = [v_p.tile([P, DA * H], BF16, name=f"vA{t}") for t in range(NT)]
            yTp = [yT_p.tile([P, T], F32R, name=f"yTp{cb}") for cb in range(NCB)]
            # DRAM staging for the reciprocal rows (SBUF->SBUF DMA cannot do
            # 0-stride broadcast, DRAM->SBUF can)
            rc_dram = nc.dram_tensor("rc_stage", [H, T], F32,
                                     kind="Internal").ap()

            with tc.tile_pool(name="xT", bufs=1) as xT_p, \
                 tc.tile_pool(name="wqk", bufs=1) as wqk_p:
                xT = [xT_p.tile([P, T], F32R, name=f"xT{cb}") for cb in range(NCB)]

                # ---- x^T transposes + v projection (scoped W_v / x loads) ----
                with tc.tile_pool(name="xl", bufs=4) as x_p, \
                     tc.tile_pool(name="wv", bufs=1) as wv_p:
                    # DMA issue order = sync-queue order: first x (gates the
                    # PE transposes), then W_v, then the bias broadcasts /
                    # scatter, then W_qk / W_proj below.
                    x_ts = []
                    for tb in range(4):
                        x_t = x_p.tile([P, C], F32, name="x_t")
                        nc.sync.dma_start(x_t[:], x_d[tb * P:(tb + 1) * P, :])
                        x_ts.append(x_t)
                    wv = []
                    for cb in range(NCB):
                        w = wv_p.tile([P, C], F32R, name=f"wv{cb}")
                        nc.sync.dma_start(w[:], wa_d[cb * P:(cb + 1) * P,
                                                     2 * C:3 * C].bitcast(F32R))
                        wv.append(w)
                    nc.sync.dma_start(
                        bv_bc[:],
                        ba_d.unsqueeze(0)[:, 2 * C:3 * C].partition_broadcast(P).squeeze(1))
                    for tb in range(4, NT):
                        x_t = x_p.tile([P, C], F32, name="x_t")
                        nc.sync.dma_start(x_t[:], x_d[tb * P:(tb + 1) * P, :])
                        x_ts.append(x_t)
                    nc.sync.dma_start(b_qk,
                                      ba_d.rearrange("(a b) -> b a", b=D)[:, 0:16])

                    # constants built while DMAs stream.  The tri mask is
                    # built in f32 (gpsimd affine_select) and rounded to the
                    # bf16 tile with a DVE copy.
                    make_identity(nc, ident)
                    tri_f = x_p.tile([P, P], F32, name="tri_f",
                                     tag="trif", bufs=1)
                    nc.gpsimd.memset(tri_f, 1.0)
                    nc.gpsimd.affine_select(
                        out=tri_f, in_=tri_f, compare_op=is_ge, fill=0.0,
                        base=0, pattern=[[1, P]], channel_multiplier=-1)
                    nc.vector.tensor_copy(tri, tri_f[:])
                    nc.vector.memset(ones8, 1.0)

                    # dummy transposes keep the PE busy while x streams in so
                    # the HAM clock-gate is already released (2.4 GHz) when
                    # the real work starts
                    for w_i in range(10):
                        warm_ps = ps.tile([P, 512], F32, name="warm_ps",
                                          tag="big", bufs=3)
                        for k in range(4):
                            nc.tensor.transpose(warm_ps[:, k * P:(k + 1) * P],
                                                ident, ident)

                    for jt in range(2):
                        for cb in range(NCB):
                            tr_ps = ps.tile([P, 512], F32, name="tr_ps",
                                            tag="big", bufs=3)
                            for k in range(4):
                                nc.tensor.transpose(
                                    tr_ps[:, k * P:(k + 1) * P],
                                    x_ts[4 * jt + k][:, cb * P:(cb + 1) * P],
                                    ident)
                            nc.vector.tensor_copy(xT[cb][:, jt * 512:(jt + 1) * 512],
                                                  tr_ps[:])
                        # v projection for this half's token blocks
                        for tb in range(4 * jt, 4 * jt + 4):
                            v_ps = ps.tile([P, C], F32, name="v_ps", tag="big", bufs=3)
                            for cb in range(NCB):
                                lhsT = xT[cb][:, tb * P:(tb + 1) * P]
                                nc.tensor.matmul(v_ps[:, 0:512], lhsT,
                                                 wv[cb][:, 0:512],
                                                 start=(cb == 0), stop=(cb == NCB - 1))
                                nc.tensor.matmul(v_ps[:, 512:C], lhsT,
                                                 wv[cb][:, 512:C],
                                                 start=(cb == 0), stop=(cb == NCB - 1))
                            for h in range(H):
                                nc.vector.tensor_add(vA[tb][:, DA * h:DA * h + D],
                                                     v_ps[:, D * h:D * h + D],
                                                     bv_bc[:, D * h:D * h + D])
                            # ones columns at local col 96 of each head's group
                            nc.vector.tensor_copy(vA[tb][:, D::DA], ones8)

                # ---- per-head attention ----
                wqk = []
                for cb in range(NCB):
                    w = wqk_p.tile([P, 2 * C], F32R, name=f"wqk{cb}")
                    nc.sync.dma_start(w[:], wa_d[cb * P:(cb + 1) * P,
                                                 0:2 * C].bitcast(F32R))
                    wqk.append(w)
                # W_proj loads go into the space freed by the wv/xl pools so
                # they complete long before the projection needs them
                wp_p = tc.alloc_tile_pool(name="wp", bufs=1)
                bp_bc = wp_p.tile([P, C], F32, name="bp_bc", tag="bpbc", bufs=1)
                nc.sync.dma_start(
                    bp_bc[:], bp_d.unsqueeze(0).partition_broadcast(P).squeeze(1))
                wp = []
                for cb in range(NCB):
                    w = wp_p.tile([P, C], F32R, name=f"wp{cb}")
                    nc.sync.dma_start(w[:], wp_d[cb * P:(cb + 1) * P, :].bitcast(F32R))
                    wp.append(w)

                Identity = mybir.ActivationFunctionType.Identity

                def emit_qkproj(h, qT, kT):
                    # qT/kT = (x @ W_{q,k} + b)^T in [d, token] bf16 layout.
                    # The q copyback runs on the DVE, the k copyback on the
                    # scalar engine (Identity + per-partition bias): balances
                    # the two engines' per-head load.
                    for dst, off, bcol, eng in (
                            (qT, D * h, b_qk[:, h:h + 1], "v"),
                            (kT, C + D * h, b_qk[:, 8 + h:9 + h], "s")):
                        qk_ps = ps.tile([D, T], F32, name="qk_ps", tag="big", bufs=3)
                        for jt in range(2):
                            sl = slice(jt * 512, (jt + 1) * 512)
                            for cb in range(NCB):
                                nc.tensor.matmul(qk_ps[:, sl],
                                                 wqk[cb][:, off:off + D],
                                                 xT[cb][:, sl],
                                                 start=(cb == 0), stop=(cb == NCB - 1))
                        if eng == "v":
                            nc.vector.tensor_scalar_add(dst[:], qk_ps[:], bcol)
                        else:
                            nc.scalar.activation(dst[:], qk_ps[:], Identity,
                                                 bias=bcol)

                def emit_tail(h, y_sb, bc_sb):
                    # part B for head h: reciprocal of the broadcast
                    # denominators (partition-parallel), normalize, scatter
                    # head rows into the feature-packed yT tiles (partition
                    # shift -> must go through DMA)
                    rcp = sm_p.tile([D, T], F32, name="rcp", tag="rcp", bufs=2)
                    nc.vector.reciprocal_approx_fast(rcp[:], bc_sb[:])
                    y_n = sm_p.tile([D, T], F32R, name="y_n", tag="yn", bufs=2)
                    nc.vector.tensor_mul(y_n[:], y_sb[0:D, :], rcp[:])
                    f0 = D * h
                    while f0 < D * (h + 1):
                        cb2, b0 = f0 // P, f0 % P
                        seg = min(P - b0, D * (h + 1) - f0)
                        nc.gpsimd.dma_start(
                            yTp[cb2][b0:b0 + seg, :],
                            y_n[f0 - D * h:f0 - D * h + seg, :])
                        f0 += seg

                qT = qk_p.tile([D, T], BF16, name="qT", tag="qkt")
                kT = qk_p.tile([D, T], BF16, name="kT", tag="qkt")
                emit_qkproj(0, qT, kT)

                pending = None
                for h in range(H):
                    # ---- S^T blocks + exp ----
                    ptiles = []
                    for ib in range(NT):
                        q0 = P * ib
                        s_ps = ps.tile([P, T], F32, name="s_ps", tag="big", bufs=3)
                        kblk = kT[:, ib * P:(ib + 1) * P]
                        if q0 < 512:
                            nc.tensor.matmul(s_ps[:, q0:512], kblk,
                                             qT[:, q0:512], start=True, stop=True)
                        r0 = max(q0, 512)
                        nc.tensor.matmul(s_ps[:, r0:T], kblk,
                                         qT[:, r0:T], start=True, stop=True)
                        p_t = p_p.tile([P, T], BF16, name="p_t")
                        nc.scalar.activation(p_t[:, q0:T], s_ps[:, q0:T],
                                             Exp, scale=SCALE)
                        # zero the upper triangle of the diagonal 128-col block
                        nc.vector.tensor_mul(p_t[:, q0:q0 + P],
                                             p_t[:, q0:q0 + P], tri)
                        ptiles.append(p_t)

                    # ---- next head's q/k projection: PE work that overlaps
                    # the exp chain above ----
                    if h + 1 < H:
                        qT_n = qk_p.tile([D, T], BF16, name="qT", tag="qkt")
                        kT_n = qk_p.tile([D, T], BF16, name="kT", tag="qkt")
                        emit_qkproj(h + 1, qT_n, kT_n)

                    # ---- P@V with causal N-restriction; two bank-halves of
                    # q, each its own accumulation group.  Each half's tail
                    # part A (stage to SBUF + DRAM roundtrip for the
                    # partition broadcast) is emitted as soon as that half's
                    # accumulation group closes, so the last head's
                    # normalization chain starts mid-PV instead of after it.
                    # The reciprocal + normalization (part B) are deferred one
                    # head so no DVE op waits on the DMA roundtrip at the
                    # FIFO head. ----
                    y_l = ps.tile([DA, 512], F32, name="y_l", tag="yps", bufs=2)
                    y_r = ps.tile([DA, 512], F32, name="y_r", tag="yps", bufs=2)
                    y_sb = sm_p.tile([DA, T], F32, name="y_sb", tag="ysb", bufs=2)
                    bc_sb = sm_p.tile([D, T], F32, name="bc_sb", tag="bc", bufs=2)
                    row = rc_dram[h:h + 1, :]
                    for ib in range(NT):
                        q0 = P * ib
                        va = vA[ib][:, DA * h:DA * h + DA]
                        if q0 < 512:
                            nc.tensor.matmul(y_l[:, q0:512], va,
                                             ptiles[ib][:, q0:512],
                                             start=(ib == 0), stop=(ib == 3))
                            nc.tensor.matmul(y_r[:], va, ptiles[ib][:, 512:T],
                                             start=(ib == 0), stop=False)
                        else:
                            nc.tensor.matmul(y_r[:, q0 - 512:512], va,
                                             ptiles[ib][:, q0:T],
                                             start=False, stop=(ib == NT - 1))
                    nc.vector.tensor_copy(y_sb[:, 0:512], y_l[:])
                    nc.vector.tensor_copy(y_sb[:, 512:T], y_r[:])
                    # tail DMAs go through the (otherwise idle) gpsimd queue:
                    # the store->broadcast dependency would head-block the
                    # sync queue for the whole PV duration otherwise
                    nc.gpsimd.dma_start(row, y_sb[D:DA, :])
                    nc.gpsimd.dma_start(bc_sb[:],
                                        row.partition_broadcast(D).squeeze(1))
                    if pending is not None:
                        emit_tail(*pending)
                    pending = (h, y_sb, bc_sb)
                    if h + 1 < H:
                        qT, kT = qT_n, kT_n

                # ---------------- projection ----------------
                # The first three token blocks emit their first 4
                # accumulation matmuls (features 0:512 = heads 0-5, long
                # since scattered) before the last head's part-B tail, so its
                # DMA roundtrip is covered by PE work; the cb4/cb5 matmuls
                # (heads 6-7 features) follow after.
                NEARLY = 3
                o_pss = []
                for tb in range(NEARLY):
                    o_ps = ps.tile([P, C], F32, name="o_ps", tag="big", bufs=3)
                    for cb in range(4):
                        nc.tensor.matmul(o_ps[:, 0:512],
                                         yTp[cb][:, tb * P:(tb + 1) * P],
                                         wp[cb][:, 0:512],
                                         start=(cb == 0), stop=False)
                    for cb in range(4):
                        nc.tensor.matmul(o_ps[:, 512:C],
                                         yTp[cb][:, tb * P:(tb + 1) * P],
                                         wp[cb][:, 512:C],
                                         start=(cb == 0), stop=False)
                    o_pss.append(o_ps)
                if pending is not None:
                    emit_tail(*pending)
                    pending = None
                for tb in range(NT):
                    if tb < NEARLY:
                        o_ps = o_pss[tb]
                        for cb in range(4, NCB):
                            nc.tensor.matmul(o_ps[:, 0:512],
                                             yTp[cb][:, tb * P:(tb + 1) * P],
                                             wp[cb][:, 0:512],
                                             start=False, stop=(cb == NCB - 1))
                        o_sb = o_p.tile([P, C], F32, name="o_sb")
                        for cb in range(4, NCB):
                            nc.tensor.matmul(o_ps[:, 512:C],
                                             yTp[cb][:, tb * P:(tb + 1) * P],
                                             wp[cb][:, 512:C],
                                             start=False, stop=(cb == NCB - 1))
                    else:
                        o_ps = ps.tile([P, C], F32, name="o_ps", tag="big", bufs=3)
                        for cb in range(NCB):
                            nc.tensor.matmul(o_ps[:, 0:512],
                                             yTp[cb][:, tb * P:(tb + 1) * P],
                                             wp[cb][:, 0:512],
                                             start=(cb == 0), stop=(cb == NCB - 1))
                        o_sb = o_p.tile([P, C], F32, name="o_sb")
                        for cb in range(NCB):
                            nc.tensor.matmul(o_ps[:, 512:C],
                                             yTp[cb][:, tb * P:(tb + 1) * P],
                                             wp[cb][:, 512:C],
                                             start=(cb == 0), stop=(cb == NCB - 1))
                    nc.vector.tensor_add(o_sb[:, 0:512], o_ps[:, 0:512],
                                         bp_bc[:, 0:512])
                    nc.sync.dma_start(out_d[tb * P:(tb + 1) * P, 0:512],
                                      o_sb[:, 0:512])
                    nc.vector.tensor_add(o_sb[:, 512:C], o_ps[:, 512:C],
                                         bp_bc[:, 512:C])
                    nc.sync.dma_start(out_d[tb * P:(tb + 1) * P, 512:C],
                                      o_sb[:, 512:C])
                wp_p.release()

    # The act-table-load pass assigns each activation the first table set
    # containing its function; Exp and Identity must land in the same set or
    # every qk-copyback/exp switch costs a 1.3us table reload.  All our
    # functions live together in natural_log_exp_and_others, so hide the
    # other sets (keeping dict order — act_func_set_id is positional) during
    # this build.
    import concourse.hw_specs as hw_specs
    orig_tables = hw_specs.get_activation_tables

    def _tables(arch, *a, **kw):
        tabs = orig_tables(arch, *a, **kw)
        pref = "natural_log_exp_and_others"
        if pref not in tabs:
            return tabs
        return {k: (v if k == pref else type(v)()) for k, v in tabs.items()}

    import concourse.bacc as bacc_mod
    hw_specs.get_activation_tables = _tables
    bacc_orig = getattr(bacc_mod, "get_activation_tables", None)
    try:
        if bacc_orig is not None:
            bacc_mod.get_activation_tables = _tables
        nc.compile()
    finally:
        hw_specs.get_activation_tables = orig_tables
        if bacc_orig is not None:
            bacc_mod.get_activation_tables = bacc_orig
    return nc


def run(inputs, trace=False):
    import concourse.bass_utils as bass_utils

    nc = _CACHE.get("nc")
    if nc is None:
        nc = _CACHE["nc"] = _build()

    x = np.ascontiguousarray(inputs["x"], dtype=np.float32)
    wa = np.ascontiguousarray(inputs["W_attn"], dtype=np.float32)
    ba = np.ascontiguousarray(inputs["b_attn"], dtype=np.float32)
    wp = np.ascontiguousarray(inputs["W_proj"], dtype=np.float32)
    bp = np.ascontiguousarray(inputs["b_proj"], dtype=np.float32)
    B = x.shape[0]
    in_maps = [
        {"x": np.ascontiguousarray(x[b]), "W_attn": wa, "b_attn": ba,
         "W_proj": wp, "b_proj": bp}
        for b in range(B)
    ]
    res = bass_utils.run_bass_kernel_spmd(
        nc, in_maps, core_ids=list(range(B)), trace=trace)
    out = np.stack([r["out"] for r in res.results], axis=0)
    return out, res


def kernel(**inputs):
    out, _ = run(inputs, trace=False)
    return out
